# revision 2
# baseline (speedup 1.0000x reference)
"""Trainium2 Bass kernel: sliding-window multi-head attention with ALiBi.

Reference computation (B=2, S=4096, E=512, H=8, D=64, window 513):
    q = (inputs_q @ w_q);  k = (inputs_kv @ w_k);  v = (inputs_kv @ w_v)
    att = softmax(q k^T / 8 + alibi, sliding window +-256)
    out = (att v) @ w_o

Sharding: 8 cores = 2 batches x 4 sequence quarters (1024 q rows per core),
kv slices carry a 256-row zero-padded halo; a host-provided validity column
appended to V makes the softmax denominator (accumulated by the AV matmul)
skip padded rows; the window/ALiBi mask is a multiplicative exp-mask applied
after exp() (G = exp(-slope_h*|rel|) * [|rel|<=256], Toeplitz, shared by all
q blocks).

Scores are computed transposed (S^T[kv, q]) so AV needs no transposes:
lhsT = [V | valid], rhs = P^T gives O^T[d, q] plus the denominator row.

v3 structure (vs the 105us two-phase baseline): there is no separate
projection phase.  A short preamble projects only what the first attention
iterations need (q/k head-pair 3, V blocks 0-7); the remaining 19 projection
matmul groups are interleaved into the attention stream (<=2 per iteration)
where they fill PE idle slots while the DVE/ACT-bound softmax chain runs.
  - merged score layout: one [128, 1280] PSUM tile (3 banks) per head;
    ONE exp and ONE mask-mul per head.
  - head pairing: heads (2t+1, 2t) are processed together, score matmuls
    interleaved; even heads on PE row group 0, odd heads on row group 64
    (K=64), so adjacent score matmuls run concurrently in the PE array.
  - hp-major iteration order (all q blocks of a head pair, then the next
    pair) so interleaved q/k projections are produced 2+ iterations before
    their first use.
  - shared AV-output bank per pair: one [65, 512] PSUM tile, one reciprocal
    and one GPSIMD partition-broadcast per pair.
  - early-tail emission: block order is scores(i) / tail(i-2) / chain(i-1) /
    proj(i), so the tail's norm frees the shared 1-bank PSUM slot before the
    interleaved projection needs it.
  - PSUM: 2x3 banks (score tiles) + 2x1 bank (AV-pair / y-proj / interleaved
    projections, one shared tag) = 8 banks exactly.
  - engine placement: exp on ACT, mask-mul + reciprocal + normalize + k/v
    evictions on DVE, q evictions + V-copy + y staging on ACT,
    partition-broadcast on GPSIMD.
"""

import sys

if "/opt/trn_rl_repo" not in sys.path:
    sys.path.insert(0, "/opt/trn_rl_repo")

import numpy as np

import concourse.bacc as bacc
import concourse.mybir as mybir
import concourse.tile as tile
from concourse.bass_utils import run_bass_kernel_spmd

# ---------------------------------------------------------------- geometry
B, S, E = 2, 4096, 512
H, D = 8, 64
HD = H * D              # 512
HALF = 256              # window half-width (ATTENTION_WINDOW=512 -> 513 wide)
NCORES = 8
SQ = 4                  # sequence shards per batch
QROWS = S // SQ         # 1024 q rows per core
KVROWS = QROWS + 2 * HALF   # 1536 kv rows per core (with halo)
QB = 4                  # q blocks per core
QBLK = QROWS // QB      # 256 q cols per block
NCH = 6                 # kv chunks per q block
CBLK = 128              # kv chunk rows

F32 = mybir.dt.float32
BF16 = mybir.dt.bfloat16

GTOT = 1280             # exp'd score cols per head-iteration

# Merged score layout: one 3-bank PSUM tile [128, 1280] per head.
# Bank boundaries at 512/1024 cols: every piece stays inside one bank.
CH_GOFF = {1: 0, 2: 256, 3: 512, 0: 768, 5: 768, 4: 1024}
CH_LO = {0: 0, 1: 0, 2: 0, 3: 0, 4: 0, 5: 128}
CH_HI = {0: 128, 1: 256, 2: 256, 3: 256, 4: 256, 5: 256}
# emission order with start=True on the first piece touching each PSUM bank
CH_EMIT = [(1, True), (2, False), (3, True), (0, False), (5, False),
           (4, True)]
CH_OFF = CH_GOFF

_CACHE = {}


def _build_program(repeats=1):
    """Build + compile the SPMD program (cached per process)."""
    key = ("nc", repeats)
    if key in _CACHE:
        return _CACHE[key]

    nc = bacc.Bacc("TRN2", target_bir_lowering=False, debug=False,
                   enable_asserts=True)

    xq_d = nc.dram_tensor("xqT", [E, QROWS], BF16, kind="ExternalInput")
    xkv_d = nc.dram_tensor("xkvT", [E, KVROWS], BF16, kind="ExternalInput")
    wq_d = nc.dram_tensor("wq", [E, HD], BF16, kind="ExternalInput")
    wk_d = nc.dram_tensor("wk", [E, HD], BF16, kind="ExternalInput")
    wv_d = nc.dram_tensor("wv", [E, HD], BF16, kind="ExternalInput")
    wo_d = nc.dram_tensor("wo", [HD, E], BF16, kind="ExternalInput")
    g_d = nc.dram_tensor("gmask", [H, 128, GTOT], BF16, kind="ExternalInput")
    val_d = nc.dram_tensor("validc", [128, KVROWS // CBLK], F32,
                           kind="ExternalInput")
    y_d = nc.dram_tensor("y", [QROWS, E], F32, kind="ExternalOutput")

    EXP = mybir.ActivationFunctionType.Exp

    with tile.TileContext(nc) as tc:
        with (
            tc.tile_pool(name="wts", bufs=16) as wts,
            tc.tile_pool(name="gms", bufs=8) as gms,
            tc.tile_pool(name="bigx", bufs=8) as bigx,
            tc.tile_pool(name="proj", bufs=1) as proj,
            tc.tile_pool(name="pwork", bufs=3) as pwork,
            tc.tile_pool(name="small", bufs=4) as small,
            tc.tile_pool(name="spw0", bufs=2, space="PSUM") as spw0,
            tc.tile_pool(name="spw1", bufs=2, space="PSUM") as spw1,
        ):
            def alloc4(name, cols):
                return [wts.tile([128, cols], BF16, tag="w",
                                 name=f"{name}{e}") for e in range(4)]

            def dma4(ts, dram):
                for e in range(4):
                    nc.sync.dma_start(ts[e][:],
                                      dram.ap()[128 * e:128 * (e + 1), :])

            wq_sb = alloc4("wq", HD)
            wk_sb = alloc4("wk", HD)
            wv_sb = alloc4("wv", HD)
            wo_sb = alloc4("wo", E)

            # persistent Toeplitz exp-masks (loaded once, like the weights)
            g_sb = [gms.tile([128, GTOT], BF16, tag="g", name=f"g{h}")
                    for h in range(H)]

            valid_sb = small.tile([128, KVROWS // CBLK], F32, tag="validc",
                                  name="validc")
            ones8 = small.tile([128, H], F32, tag="ones8", name="ones8")
            nc.vector.memset(ones8[:], 1.0)

            # ---- persistent activation tiles
            qt_sb = [proj.tile([128, QROWS], BF16, tag=f"qt{t}", name=f"qt{t}")
                     for t in range(4)]
            kt_sb = [proj.tile([128, KVROWS], BF16, tag=f"kt{t}",
                               name=f"kt{t}") for t in range(4)]
            # V tiles: head h occupies cols [65h, 65h+64), col 65h+64 = valid
            v_sb = [proj.tile([128, 65 * H], BF16, tag=f"v{b}", name=f"v{b}")
                    for b in range(KVROWS // CBLK)]
            ot_sb = [proj.tile([128, QROWS], BF16, tag=f"ot{t}", name=f"ot{t}")
                     for t in range(4)]

            for rep in range(repeats):
                if rep == 0:
                    dma4(wq_sb, wq_d)
                xq_sb = []
                for e in range(4):
                    t = bigx.tile([128, KVROWS], BF16, tag="bigx",
                                  name=f"xq{e}")
                    nc.sync.dma_start(t[:, :QROWS],
                                      xq_d.ap()[128 * e:128 * (e + 1), :])
                    xq_sb.append(t)
                if rep == 0:
                    dma4(wk_sb, wk_d)
                xkv_sb = []
                for e in range(4):
                    t = bigx.tile([128, KVROWS], BF16, tag="bigx",
                                  name=f"xkv{e}")
                    nc.sync.dma_start(t[:],
                                      xkv_d.ap()[128 * e:128 * (e + 1), :])
                    xkv_sb.append(t)
                if rep == 0:
                    dma4(wv_sb, wv_d)
                    nc.sync.dma_start(valid_sb[:], val_d.ap()[:])
                    for h in range(H):
                        nc.sync.dma_start(g_sb[h][:], g_d.ap()[h])
                    dma4(wo_sb, wo_d)

                # ---- projection groups (emitted piecemeal, see schedule)
                def qproj_group(t, n, pool, tag):
                    ps = pool.tile([128, 512], F32, tag=tag, name="psq")
                    for e in range(4):
                        nc.tensor.matmul(
                            ps[:],
                            lhsT=wq_sb[e][:, 128 * t:128 * (t + 1)],
                            rhs=xq_sb[e][:, 512 * n:512 * (n + 1)],
                            start=(e == 0), stop=(e == 3))
                    nc.scalar.copy(
                        qt_sb[t][:, 512 * n:512 * (n + 1)], ps[:])

                def kproj_group(t, n, pool, tag):
                    ps = pool.tile([128, 512], F32, tag=tag, name="psk")
                    for e in range(4):
                        nc.tensor.matmul(
                            ps[:],
                            lhsT=wk_sb[e][:, 128 * t:128 * (t + 1)],
                            rhs=xkv_sb[e][:, 512 * n:512 * (n + 1)],
                            start=(e == 0), stop=(e == 3))
                    nc.vector.tensor_copy(
                        kt_sb[t][:, 512 * n:512 * (n + 1)], ps[:])

                def vproj_group(blk, pool, tag):
                    ps = pool.tile([128, 512], F32, tag=tag, name="psv")
                    for e in range(4):
                        nc.tensor.matmul(
                            ps[:],
                            lhsT=xkv_sb[e][:, 128 * blk:128 * (blk + 1)],
                            rhs=wv_sb[e][:],
                            start=(e == 0), stop=(e == 3))
                    vv = v_sb[blk][:].rearrange("p (h c) -> p h c", c=65)
                    nc.scalar.copy(
                        vv[:, :, 0:64],
                        ps[:].rearrange("p (h c) -> p h c", c=64))
                    nc.vector.tensor_scalar_mul(
                        vv[:, :, 64], ones8[:],
                        valid_sb[:, blk:blk + 1])

                # preamble: what iterations 0-3 need (hp=3, V blocks 0-7)
                qproj_group(3, 0, spw0, "sp")
                kproj_group(3, 0, spw0, "sp")
                kproj_group(3, 1, spw0, "sp")
                qproj_group(3, 1, spw0, "sp")
                kproj_group(3, 2, spw0, "sp")
                for blk in range(8):
                    vproj_group(blk, spw0, "sp")

                # interleave schedule: projection groups per iteration block,
                # emitted AFTER chain(i-1) into the shared 1-bank PSUM tag.
                # Every group lands >=1 block before its first reader.
                INTER = {
                    0: [("q", 2, 0), ("q", 2, 1)],
                    1: [("v", 8), ("k", 2, 0)],
                    2: [("v", 9), ("v", 10)],
                    3: [("v", 11), ("k", 2, 1)],
                    4: [("k", 2, 2), ("q", 1, 0)],
                    5: [("q", 1, 1), ("k", 1, 0)],
                    6: [("k", 1, 1), ("k", 1, 2)],
                    7: [("q", 0, 0), ("q", 0, 1)],
                    8: [("k", 0, 0), ("k", 0, 1)],
                    9: [("k", 0, 2)],
                }

                def emit_proj(blocki):
                    for g in INTER.get(blocki, ()):
                        if g[0] == "q":
                            qproj_group(g[1], g[2], spw1, "spb")
                        elif g[0] == "k":
                            kproj_group(g[1], g[2], spw1, "spb")
                        else:
                            vproj_group(g[1], spw1, "spb")

                # ---- attention, software-pipelined at head-PAIR granularity
                # (16 iterations = 4 head pairs x 4 q blocks, hp-major).
                def emit_scores(qb, hp, sp):
                    # head B=2hp+1 on PE rows 64:128, head A=2hp on rows
                    # 0:64 -> adjacent matmuls run concurrently (row groups)
                    th = hp
                    qsB = qt_sb[th][64:128, QBLK * qb:QBLK * (qb + 1)]
                    qsA = qt_sb[th][0:64, QBLK * qb:QBLK * (qb + 1)]
                    for c, st in CH_EMIT:
                        k0 = QBLK * qb + CBLK * c
                        o = CH_GOFF[c]
                        lo, hi = CH_LO[c], CH_HI[c]
                        nc.tensor.matmul(
                            sp[1][:, o + lo:o + hi],
                            lhsT=kt_sb[th][64:128, k0:k0 + CBLK],
                            rhs=qsB[:, lo:hi],
                            start=st, stop=True,
                            skip_group_check=True)
                        nc.tensor.matmul(
                            sp[0][:, o + lo:o + hi],
                            lhsT=kt_sb[th][0:64, k0:k0 + CBLK],
                            rhs=qsA[:, lo:hi],
                            start=st, stop=True,
                            skip_group_check=True)

                def emit_chain(qb, hp, sp):
                    # per head: exp -> mask-mul -> AV (B first)
                    ot = spw1.tile([65, 512], F32, tag="spb", name="ot")
                    for j, side in ((1, 0), (0, 256)):
                        h = 2 * hp + j
                        pe = pwork.tile([128, GTOT], BF16, tag=f"pe{j}",
                                        name=f"pe{j}")
                        nc.scalar.activation(pe[:], sp[j][:], EXP)
                        pm = pwork.tile([128, GTOT], BF16, tag=f"pm{j}",
                                        name=f"pm{j}")
                        nc.vector.tensor_mul(pm[:], pe[:], g_sb[h][:])
                        for i, (c, _) in enumerate(CH_EMIT):
                            o = CH_GOFF[c]
                            lo, hi = CH_LO[c], CH_HI[c]
                            nc.tensor.matmul(
                                ot[:, side + lo:side + hi],
                                lhsT=v_sb[2 * qb + c][:, 65 * h:65 * h + 65],
                                rhs=pm[:, o + lo:o + hi],
                                start=(i == 0), stop=(i == len(CH_EMIT) - 1),
                                skip_group_check=(i > 0))
                    rec = small.tile([1, 512], F32, tag="rec", name="rec")
                    nc.vector.reciprocal(rec[:], ot[64:65, :])
                    return (qb, hp, ot, rec)

                def emit_tail(qb, hp, ot, rec):
                    th = hp
                    bc = pwork.tile([64, 512], F32, tag="bc", name="bc")
                    nc.gpsimd.partition_broadcast(bc[:], rec[:])
                    # head B=2hp+1 -> ot_sb rows 64:128, head A -> rows 0:64
                    nc.vector.tensor_mul(
                        ot_sb[th][64:128, QBLK * qb:QBLK * (qb + 1)],
                        ot[0:64, 0:256], bc[:, 0:256])
                    nc.vector.tensor_mul(
                        ot_sb[th][0:64, QBLK * qb:QBLK * (qb + 1)],
                        ot[0:64, 256:512], bc[:, 256:512])
                    if hp == 0:
                        emit_yproj(qb)

                def emit_yproj(qb):
                    for yb in (2 * qb, 2 * qb + 1):
                        yp = spw1.tile([128, 512], F32, tag="spb", name="yp")
                        for t in range(4):
                            nc.tensor.matmul(
                                yp[:],
                                lhsT=ot_sb[t][:, 128 * yb:128 * (yb + 1)],
                                rhs=wo_sb[t][:],
                                start=(t == 0), stop=(t == 3))
                        ys = pwork.tile([128, 512], F32, tag="ys",
                                        name="ystage")
                        nc.scalar.copy(ys[:], yp[:])
                        nc.sync.dma_start(
                            y_d.ap()[128 * yb:128 * (yb + 1), :], ys[:])

                pend_chain = None
                pend_tail = None
                blocki = 0
                for hp in range(H // 2 - 1, -1, -1):
                    for qb in range(QB):
                        sp = [
                            spw0.tile([128, GTOT], F32, tag="sp",
                                      name="spA"),
                            spw0.tile([128, GTOT], F32, tag="sp",
                                      name="spB"),
                        ]
                        emit_scores(qb, hp, sp)
                        if pend_tail is not None:
                            emit_tail(*pend_tail)
                        done = (emit_chain(*pend_chain)
                                if pend_chain is not None else None)
                        pend_tail = done
                        pend_chain = (qb, hp, sp)
                        emit_proj(blocki)
                        blocki += 1
                done = emit_chain(*pend_chain)
                if pend_tail is not None:
                    emit_tail(*pend_tail)
                emit_tail(*done)

    nc.compile()
    _CACHE[key] = nc
    return nc


def build_in_maps(inputs_q, inputs_kv, w_q, w_k, w_v, w_o):
    """Host-side sharding: slice/transpose/pad per core + mask tensors."""
    np_bf = mybir.dt.np(BF16)
    inputs_q = np.asarray(inputs_q, np.float32)
    inputs_kv = np.asarray(inputs_kv, np.float32)

    wq = np.ascontiguousarray(np.asarray(w_q, np.float32) * 0.125).astype(np_bf)
    wk = np.ascontiguousarray(np.asarray(w_k, np.float32)).astype(np_bf)
    wv = np.ascontiguousarray(np.asarray(w_v, np.float32)).astype(np_bf)
    wo = np.ascontiguousarray(np.asarray(w_o, np.float32)).astype(np_bf)

    # Toeplitz exp-mask, pre-unrolled into the reordered score layout
    # (chunk c at col offset CH_OFF[c]; chunks 0 and 5 overlay one block
    # with disjoint support): rel = i - r - 128c + 256
    slopes = np.array([2.0 ** (-(i + 1)) for i in range(H)], np.float64)
    r = np.arange(128)[:, None]
    i = np.arange(QBLK)[None, :]
    g32 = np.zeros((H, 128, GTOT), np.float32)
    for c in range(NCH):
        rel = i - r - 128 * c + 256
        band = (np.abs(rel) <= HALF)
        off = CH_OFF[c]
        for h in range(H):
            g32[h, :, off:off + QBLK] += (
                np.exp(-slopes[h] * np.abs(rel)) * band).astype(np.float32)
    g = g32.astype(np_bf)

    in_maps = []
    for c in range(NCORES):
        b, sq = divmod(c, SQ)
        g0 = QROWS * sq
        xq = np.ascontiguousarray(
            inputs_q[b, g0:g0 + QROWS, :].T).astype(np_bf)
        kvlo = g0 - HALF
        lo, hi = max(0, kvlo), min(S, g0 + QROWS + HALF)
        xkv = np.zeros((E, KVROWS), np_bf)
        xkv[:, lo - kvlo:hi - kvlo] = inputs_kv[b, lo:hi, :].T.astype(np_bf)
        valid = np.zeros((KVROWS,), np.float32)
        valid[lo - kvlo:hi - kvlo] = 1.0
        validc = np.ascontiguousarray(valid.reshape(KVROWS // CBLK, CBLK).T)
        in_maps.append({
            "xqT": xq, "xkvT": xkv,
            "wq": wq, "wk": wk, "wv": wv, "wo": wo,
            "gmask": g, "validc": validc,
        })
    return in_maps


def assemble_output(results):
    out = np.empty((B, S, E), np.float32)
    for c in range(NCORES):
        b, sq = divmod(c, SQ)
        out[b, QROWS * sq:QROWS * (sq + 1), :] = results[c]["y"]
    return out


def kernel(inputs_q, inputs_kv, w_q, w_k, w_v, w_o):
    nc = _build_program()
    in_maps = build_in_maps(inputs_q, inputs_kv, w_q, w_k, w_v, w_o)
    res = run_bass_kernel_spmd(nc, in_maps, core_ids=list(range(NCORES)))
    return assemble_output(res.results)


# revision 3
# speedup vs baseline: 1.0828x; 1.0828x over previous
"""Trainium2 Bass kernel: sliding-window multi-head attention with ALiBi.

Reference computation (B=2, S=4096, E=512, H=8, D=64, window 513):
    q = (inputs_q @ w_q);  k = (inputs_kv @ w_k);  v = (inputs_kv @ w_v)
    att = softmax(q k^T / 8 + alibi, sliding window +-256)
    out = (att v) @ w_o

Sharding: 8 cores = 2 batches x 4 sequence quarters (1024 q rows per core),
kv slices carry a 256-row zero-padded halo; a host-provided validity column
appended to V makes the softmax denominator (accumulated by the AV matmul)
skip padded rows; the window/ALiBi mask is a multiplicative exp-mask applied
after exp() (G = exp(-slope_h*|rel|) * [|rel|<=256], Toeplitz, shared by all
q blocks).

Scores are computed transposed (S^T[kv, q]) so AV needs no transposes:
lhsT = [V | valid], rhs = P^T gives O^T[d, q] plus the denominator row.

v3 structure (vs the 105us two-phase baseline): there is no separate
projection phase.  A short preamble projects only what the first attention
iterations need (q/k head-pair 3, V blocks 0-7); the remaining 19 projection
matmul groups are interleaved into the attention stream (<=2 per iteration)
where they fill PE idle slots while the DVE/ACT-bound softmax chain runs.
  - merged score layout: one [128, 1280] PSUM tile (3 banks) per head;
    ONE exp and ONE mask-mul per head.
  - head pairing: heads (2t+1, 2t) are processed together, score matmuls
    interleaved; even heads on PE row group 0, odd heads on row group 64
    (K=64), so adjacent score matmuls run concurrently in the PE array.
  - hp-major iteration order (all q blocks of a head pair, then the next
    pair) so interleaved q/k projections are produced 2+ iterations before
    their first use.
  - shared AV-output bank per pair: one [65, 512] PSUM tile, one reciprocal
    and one GPSIMD partition-broadcast per pair.
  - early-tail emission: block order is scores(i) / tail(i-2) / chain(i-1) /
    proj(i), so the tail's norm frees the shared 1-bank PSUM slot before the
    interleaved projection needs it.
  - PSUM: 2x3 banks (score tiles) + 2x1 bank (AV-pair / y-proj / interleaved
    projections, one shared tag) = 8 banks exactly.
  - engine placement: exp on ACT, mask-mul + reciprocal + normalize + k/v
    evictions on DVE, q evictions + V-copy + y staging on ACT,
    partition-broadcast on GPSIMD.
"""

import sys

if "/opt/trn_rl_repo" not in sys.path:
    sys.path.insert(0, "/opt/trn_rl_repo")

import numpy as np

import concourse.bacc as bacc
import concourse.mybir as mybir
import concourse.tile as tile
from concourse.bass_utils import run_bass_kernel_spmd

# ---------------------------------------------------------------- geometry
B, S, E = 2, 4096, 512
H, D = 8, 64
HD = H * D              # 512
HALF = 256              # window half-width (ATTENTION_WINDOW=512 -> 513 wide)
NCORES = 8
SQ = 4                  # sequence shards per batch
QROWS = S // SQ         # 1024 q rows per core
KVROWS = QROWS + 2 * HALF   # 1536 kv rows per core (with halo)
QB = 4                  # q blocks per core
QBLK = QROWS // QB      # 256 q cols per block
NCH = 6                 # kv chunks per q block
CBLK = 128              # kv chunk rows

F32 = mybir.dt.float32
BF16 = mybir.dt.bfloat16

GTOT = 1280             # exp'd score cols per head-iteration

# Merged score layout: one 3-bank PSUM tile [128, 1280] per head.
# Bank boundaries at 512/1024 cols: every piece stays inside one bank.
CH_GOFF = {1: 0, 2: 256, 3: 512, 0: 768, 5: 768, 4: 1024}
CH_LO = {0: 0, 1: 0, 2: 0, 3: 0, 4: 0, 5: 128}
CH_HI = {0: 128, 1: 256, 2: 256, 3: 256, 4: 256, 5: 256}
# emission order with start=True on the first piece touching each PSUM bank
CH_EMIT = [(1, True), (2, False), (3, True), (0, False), (5, False),
           (4, True)]
CH_OFF = CH_GOFF

_CACHE = {}


def _build_program(repeats=1):
    """Build + compile the SPMD program (cached per process)."""
    key = ("nc", repeats)
    if key in _CACHE:
        return _CACHE[key]

    nc = bacc.Bacc("TRN2", target_bir_lowering=False, debug=False,
                   enable_asserts=True)

    xq_d = nc.dram_tensor("xqT", [E, QROWS], BF16, kind="ExternalInput")
    xkv_d = nc.dram_tensor("xkvT", [E, KVROWS], BF16, kind="ExternalInput")
    wq_d = nc.dram_tensor("wq", [E, HD], BF16, kind="ExternalInput")
    wk_d = nc.dram_tensor("wk", [E, HD], BF16, kind="ExternalInput")
    wv_d = nc.dram_tensor("wv", [E, HD], BF16, kind="ExternalInput")
    wo_d = nc.dram_tensor("wo", [HD, E], BF16, kind="ExternalInput")
    g_d = nc.dram_tensor("gmask", [H, 128, GTOT], BF16, kind="ExternalInput")
    val_d = nc.dram_tensor("validc", [128, KVROWS // CBLK], F32,
                           kind="ExternalInput")
    y_d = nc.dram_tensor("y", [QROWS, E], F32, kind="ExternalOutput")

    EXP = mybir.ActivationFunctionType.Exp

    with tile.TileContext(nc) as tc:
        with (
            tc.tile_pool(name="wts", bufs=16) as wts,
            tc.tile_pool(name="gms", bufs=8) as gms,
            tc.tile_pool(name="bigx", bufs=8) as bigx,
            tc.tile_pool(name="proj", bufs=1) as proj,
            tc.tile_pool(name="pwork", bufs=4) as pwork,
            tc.tile_pool(name="small", bufs=4) as small,
            tc.tile_pool(name="spw0", bufs=2, space="PSUM") as spw0,
            tc.tile_pool(name="spw1", bufs=2, space="PSUM") as spw1,
        ):
            def alloc4(name, cols):
                return [wts.tile([128, cols], BF16, tag="w",
                                 name=f"{name}{e}") for e in range(4)]

            def dma4(ts, dram):
                for e in range(4):
                    nc.sync.dma_start(ts[e][:],
                                      dram.ap()[128 * e:128 * (e + 1), :])

            wq_sb = alloc4("wq", HD)
            wk_sb = alloc4("wk", HD)
            wv_sb = alloc4("wv", HD)
            wo_sb = alloc4("wo", E)

            # persistent Toeplitz exp-masks (loaded once, like the weights)
            g_sb = [gms.tile([128, GTOT], BF16, tag="g", name=f"g{h}")
                    for h in range(H)]

            valid_sb = small.tile([128, KVROWS // CBLK], F32, tag="validc",
                                  name="validc")
            ones8 = small.tile([128, H], F32, tag="ones8", name="ones8")
            nc.vector.memset(ones8[:], 1.0)

            # ---- persistent activation tiles
            qt_sb = [proj.tile([128, QROWS], BF16, tag=f"qt{t}", name=f"qt{t}")
                     for t in range(4)]
            kt_sb = [proj.tile([128, KVROWS], BF16, tag=f"kt{t}",
                               name=f"kt{t}") for t in range(4)]
            # V tiles: head h occupies cols [65h, 65h+64), col 65h+64 = valid
            v_sb = [proj.tile([128, 65 * H], BF16, tag=f"v{b}", name=f"v{b}")
                    for b in range(KVROWS // CBLK)]
            ot_sb = [proj.tile([128, QROWS], BF16, tag=f"ot{t}", name=f"ot{t}")
                     for t in range(4)]

            pend_chain = None
            pend_tail = None
            for rep in range(repeats):
                if rep == 0:
                    dma4(wq_sb, wq_d)
                xq_sb = []
                for e in range(4):
                    t = bigx.tile([128, KVROWS], BF16, tag="bigx",
                                  name=f"xq{e}")
                    nc.sync.dma_start(t[:, :QROWS],
                                      xq_d.ap()[128 * e:128 * (e + 1), :])
                    xq_sb.append(t)
                if rep == 0:
                    dma4(wk_sb, wk_d)
                xkv_sb = []
                for e in range(4):
                    t = bigx.tile([128, KVROWS], BF16, tag="bigx",
                                  name=f"xkv{e}")
                    nc.sync.dma_start(t[:],
                                      xkv_d.ap()[128 * e:128 * (e + 1), :])
                    xkv_sb.append(t)
                if rep == 0:
                    dma4(wv_sb, wv_d)
                    nc.sync.dma_start(valid_sb[:], val_d.ap()[:])
                    for h in range(H):
                        nc.sync.dma_start(g_sb[h][:], g_d.ap()[h])
                    dma4(wo_sb, wo_d)

                # ---- projection groups (emitted piecemeal, see schedule)
                def qproj_group(t, n, pool, tag):
                    ps = pool.tile([128, 512], F32, tag=tag, name="psq")
                    for e in range(4):
                        nc.tensor.matmul(
                            ps[:],
                            lhsT=wq_sb[e][:, 128 * t:128 * (t + 1)],
                            rhs=xq_sb[e][:, 512 * n:512 * (n + 1)],
                            start=(e == 0), stop=(e == 3))
                    nc.scalar.copy(
                        qt_sb[t][:, 512 * n:512 * (n + 1)], ps[:])

                def kproj_group(t, n, pool, tag):
                    ps = pool.tile([128, 512], F32, tag=tag, name="psk")
                    for e in range(4):
                        nc.tensor.matmul(
                            ps[:],
                            lhsT=wk_sb[e][:, 128 * t:128 * (t + 1)],
                            rhs=xkv_sb[e][:, 512 * n:512 * (n + 1)],
                            start=(e == 0), stop=(e == 3))
                    nc.vector.tensor_copy(
                        kt_sb[t][:, 512 * n:512 * (n + 1)], ps[:])

                def vproj_group(blk, pool, tag):
                    ps = pool.tile([128, 512], F32, tag=tag, name="psv")
                    for e in range(4):
                        nc.tensor.matmul(
                            ps[:],
                            lhsT=xkv_sb[e][:, 128 * blk:128 * (blk + 1)],
                            rhs=wv_sb[e][:],
                            start=(e == 0), stop=(e == 3))
                    vv = v_sb[blk][:].rearrange("p (h c) -> p h c", c=65)
                    nc.scalar.copy(
                        vv[:, :, 0:64],
                        ps[:].rearrange("p (h c) -> p h c", c=64))
                    nc.vector.tensor_scalar_mul(
                        vv[:, :, 64], ones8[:],
                        valid_sb[:, blk:blk + 1])

                # preamble: what iterations 0-3 need (hp=3, V blocks 0-7)
                qproj_group(3, 0, spw0, "sp")
                kproj_group(3, 0, spw0, "sp")
                kproj_group(3, 1, spw0, "sp")
                qproj_group(3, 1, spw0, "sp")
                kproj_group(3, 2, spw0, "sp")
                for blk in range(8):
                    vproj_group(blk, spw0, "sp")

                # interleave schedule: projection groups per iteration block,
                # emitted AFTER chain(i-1) into the shared 1-bank PSUM tag.
                # Every group lands >=1 block before its first reader.
                INTER = {
                    0: [("q", 2, 0), ("q", 2, 1)],
                    1: [("v", 8), ("k", 2, 0)],
                    2: [("v", 9), ("v", 10)],
                    3: [("v", 11), ("k", 2, 1)],
                    4: [("k", 2, 2), ("q", 1, 0)],
                    5: [("q", 1, 1), ("k", 1, 0)],
                    6: [("k", 1, 1), ("k", 1, 2)],
                    7: [("q", 0, 0), ("q", 0, 1)],
                    8: [("k", 0, 0), ("k", 0, 1)],
                    9: [("k", 0, 2)],
                }

                def emit_proj(blocki):
                    for g in INTER.get(blocki, ()):
                        if g[0] == "q":
                            qproj_group(g[1], g[2], spw1, "spb")
                        elif g[0] == "k":
                            kproj_group(g[1], g[2], spw1, "spb")
                        else:
                            vproj_group(g[1], spw1, "spb")

                # ---- attention, software-pipelined at head-PAIR granularity
                # (16 iterations = 4 head pairs x 4 q blocks, hp-major).
                def emit_scores(qb, hp, sp):
                    # head B=2hp+1 on PE rows 64:128, head A=2hp on rows
                    # 0:64 -> adjacent matmuls run concurrently (row groups)
                    th = hp
                    qsB = qt_sb[th][64:128, QBLK * qb:QBLK * (qb + 1)]
                    qsA = qt_sb[th][0:64, QBLK * qb:QBLK * (qb + 1)]
                    for c, st in CH_EMIT:
                        k0 = QBLK * qb + CBLK * c
                        o = CH_GOFF[c]
                        lo, hi = CH_LO[c], CH_HI[c]
                        nc.tensor.matmul(
                            sp[1][:, o + lo:o + hi],
                            lhsT=kt_sb[th][64:128, k0:k0 + CBLK],
                            rhs=qsB[:, lo:hi],
                            start=st, stop=True,
                            skip_group_check=True)
                        nc.tensor.matmul(
                            sp[0][:, o + lo:o + hi],
                            lhsT=kt_sb[th][0:64, k0:k0 + CBLK],
                            rhs=qsA[:, lo:hi],
                            start=st, stop=True,
                            skip_group_check=True)

                def emit_chain(qb, hp, sp):
                    # per head: exp -> mask-mul -> AV (B first)
                    ot = spw1.tile([65, 512], F32, tag="spb", name="ot")
                    for j, side in ((1, 0), (0, 256)):
                        h = 2 * hp + j
                        pe = pwork.tile([128, GTOT], BF16, tag=f"pe{j}",
                                        name=f"pe{j}")
                        nc.scalar.activation(pe[:], sp[j][:], EXP)
                        pm = pwork.tile([128, GTOT], BF16, tag=f"pm{j}",
                                        name=f"pm{j}")
                        nc.vector.tensor_mul(pm[:], pe[:], g_sb[h][:])
                        for i, (c, _) in enumerate(CH_EMIT):
                            o = CH_GOFF[c]
                            lo, hi = CH_LO[c], CH_HI[c]
                            nc.tensor.matmul(
                                ot[:, side + lo:side + hi],
                                lhsT=v_sb[2 * qb + c][:, 65 * h:65 * h + 65],
                                rhs=pm[:, o + lo:o + hi],
                                start=(i == 0), stop=(i == len(CH_EMIT) - 1),
                                skip_group_check=(i > 0))
                    rec = small.tile([1, 512], F32, tag="rec", name="rec")
                    nc.vector.reciprocal(rec[:], ot[64:65, :])
                    return (qb, hp, ot, rec)

                def emit_tail(qb, hp, ot, rec):
                    th = hp
                    bc = pwork.tile([64, 512], F32, tag="bc", name="bc")
                    nc.gpsimd.partition_broadcast(bc[:], rec[:])
                    # head B=2hp+1 -> ot_sb rows 64:128, head A -> rows 0:64
                    nc.vector.tensor_mul(
                        ot_sb[th][64:128, QBLK * qb:QBLK * (qb + 1)],
                        ot[0:64, 0:256], bc[:, 0:256])
                    nc.vector.tensor_mul(
                        ot_sb[th][0:64, QBLK * qb:QBLK * (qb + 1)],
                        ot[0:64, 256:512], bc[:, 256:512])
                    if hp == 0:
                        emit_yproj(qb)

                def emit_yproj(qb):
                    for yb in (2 * qb, 2 * qb + 1):
                        yp = spw1.tile([128, 512], F32, tag="spb", name="yp")
                        for t in range(4):
                            nc.tensor.matmul(
                                yp[:],
                                lhsT=ot_sb[t][:, 128 * yb:128 * (yb + 1)],
                                rhs=wo_sb[t][:],
                                start=(t == 0), stop=(t == 3))
                        ys = pwork.tile([128, 512], F32, tag="ys",
                                        name="ystage")
                        nc.scalar.copy(ys[:], yp[:])
                        nc.sync.dma_start(
                            y_d.ap()[128 * yb:128 * (yb + 1), :], ys[:])

                blocki = 0
                for hp in range(H // 2 - 1, -1, -1):
                    for qb in range(QB):
                        sp = [
                            spw0.tile([128, GTOT], F32, tag="sp",
                                      name="spA"),
                            spw0.tile([128, GTOT], F32, tag="sp",
                                      name="spB"),
                        ]
                        emit_scores(qb, hp, sp)
                        if pend_tail is not None:
                            emit_tail(*pend_tail)
                        done = (emit_chain(*pend_chain)
                                if pend_chain is not None else None)
                        pend_tail = done
                        pend_chain = (qb, hp, sp)
                        emit_proj(blocki)
                        blocki += 1
                if rep == repeats - 1:
                    done = emit_chain(*pend_chain)
                    if pend_tail is not None:
                        emit_tail(*pend_tail)
                    emit_tail(*done)
                    pend_chain = None
                    pend_tail = None

    nc.compile()
    _CACHE[key] = nc
    return nc


def build_in_maps(inputs_q, inputs_kv, w_q, w_k, w_v, w_o):
    """Host-side sharding: slice/transpose/pad per core + mask tensors."""
    np_bf = mybir.dt.np(BF16)
    inputs_q = np.asarray(inputs_q, np.float32)
    inputs_kv = np.asarray(inputs_kv, np.float32)

    wq = np.ascontiguousarray(np.asarray(w_q, np.float32) * 0.125).astype(np_bf)
    wk = np.ascontiguousarray(np.asarray(w_k, np.float32)).astype(np_bf)
    wv = np.ascontiguousarray(np.asarray(w_v, np.float32)).astype(np_bf)
    wo = np.ascontiguousarray(np.asarray(w_o, np.float32)).astype(np_bf)

    # Toeplitz exp-mask, pre-unrolled into the reordered score layout
    # (chunk c at col offset CH_OFF[c]; chunks 0 and 5 overlay one block
    # with disjoint support): rel = i - r - 128c + 256
    slopes = np.array([2.0 ** (-(i + 1)) for i in range(H)], np.float64)
    r = np.arange(128)[:, None]
    i = np.arange(QBLK)[None, :]
    g32 = np.zeros((H, 128, GTOT), np.float32)
    for c in range(NCH):
        rel = i - r - 128 * c + 256
        band = (np.abs(rel) <= HALF)
        off = CH_OFF[c]
        for h in range(H):
            g32[h, :, off:off + QBLK] += (
                np.exp(-slopes[h] * np.abs(rel)) * band).astype(np.float32)
    g = g32.astype(np_bf)

    in_maps = []
    for c in range(NCORES):
        b, sq = divmod(c, SQ)
        g0 = QROWS * sq
        xq = np.ascontiguousarray(
            inputs_q[b, g0:g0 + QROWS, :].T).astype(np_bf)
        kvlo = g0 - HALF
        lo, hi = max(0, kvlo), min(S, g0 + QROWS + HALF)
        xkv = np.zeros((E, KVROWS), np_bf)
        xkv[:, lo - kvlo:hi - kvlo] = inputs_kv[b, lo:hi, :].T.astype(np_bf)
        valid = np.zeros((KVROWS,), np.float32)
        valid[lo - kvlo:hi - kvlo] = 1.0
        validc = np.ascontiguousarray(valid.reshape(KVROWS // CBLK, CBLK).T)
        in_maps.append({
            "xqT": xq, "xkvT": xkv,
            "wq": wq, "wk": wk, "wv": wv, "wo": wo,
            "gmask": g, "validc": validc,
        })
    return in_maps


def assemble_output(results):
    out = np.empty((B, S, E), np.float32)
    for c in range(NCORES):
        b, sq = divmod(c, SQ)
        out[b, QROWS * sq:QROWS * (sq + 1), :] = results[c]["y"]
    return out


def kernel(inputs_q, inputs_kv, w_q, w_k, w_v, w_o):
    nc = _build_program()
    in_maps = build_in_maps(inputs_q, inputs_kv, w_q, w_k, w_v, w_o)
    res = run_bass_kernel_spmd(nc, in_maps, core_ids=list(range(NCORES)))
    return assemble_output(res.results)
